# revision 1
# baseline (speedup 1.0000x reference)
"""Trainium2 Bass kernel for nn_AttentionSiphon.

Reference computes: tokens = x @ W_map + b_map; concat [time, cluster, tokens];
LayerNorm; per-head q/k projections; softmax(q k^T / sqrt(dh)); mean over heads;
returns rows 0 and 1 of the [B,S,S] head-mean attention.

Only attention rows 0/1 are returned, and their queries come from the
(batch-independent) time/cluster tokens. So the per-head attention collapses:

  score[j, c=2h+r] = LN(token_j) . (Wk[h] @ q_r[h])   (+ constants)

which is one [D, 34] matmul against the token matrix (columns 32/33 gather the
LN statistics' linear parts). The heavy device work is the token projection
x @ W_map ([8184,512]@[512,1024]) plus the sum of squared tokens for the LN
variance, on 8 NeuronCores with rows sharded 1024/core (tail zero-padded).

Device output per core: [34, 2, 1024] — [:,0,:] holds 32 score columns plus
col-sum and b_map-cross rows, [0,1,:] the sumsq row. The tiny softmax
epilogue ([4,16,2,2048]) runs on host.
"""

import os
import sys

sys.path.insert(0, "/opt/trn_rl_repo")

import numpy as np
import ml_dtypes

B, N, IN_D = 4, 2046, 512
D, H, DH = 1024, 16, 64
S = N + 2
EPS = 1e-5
NCORES = 8
JPC = 1024            # padded rows per core
JTOT = NCORES * JPC   # 8192 (8184 real rows + 8 pad)
NAUG = 34             # 32 score cols + colsum + b_map cross
NC_OUT = 35           # + sumsq row

# Precision scheme: "f32r" (reduced-precision fp32 matmuls at bf16 speed,
# ~2e-4 rel err), "bf16" (fastest, ~3e-3), "split" (hi/lo bf16 3-pass, ~4e-6)
PRECISION = os.environ.get("AS_PRECISION", "f32r")
WARMUP_MMS = int(os.environ.get("AS_WARM", "38"))

_PROG_CACHE = {}
LAST_RESULT = None  # BassKernelResults of the most recent run (for test harness)


def _bf16(a):
    return np.asarray(a, np.float32).astype(ml_dtypes.bfloat16)


def _split_hi_lo(a):
    a = np.asarray(a, np.float32)
    hi = a.astype(ml_dtypes.bfloat16)
    lo = (a - hi.astype(np.float32)).astype(ml_dtypes.bfloat16)
    return hi, lo


def _build_program(precision, warmup=None):
    if warmup is None:
        warmup = WARMUP_MMS
    import concourse.bacc as bacc
    import concourse.mybir as mybir
    from concourse import tile
    from concourse.tile import ScopedClock

    class LeanTailTileContext(tile.TileContext):
        """Skip the exit-path double all-engine barrier + per-sem clears.

        The kernel preamble (Bass.__init__, target_bir_lowering) already
        dma_reset+sem_clears the kernel sem range at the start of every
        execution, and this program has a single TileContext, so nothing
        downstream consumes the freed sems. The final Sync drain still
        waits on every proc (incl. DMA lanes), so outputs are complete
        before the instruction streams end.
        """

        def _drain_and_barrier(self, tick_clock, wait_clock):
            drain_inst = self.nc.sync.drain()
            wait_clock.add_sem_waits(
                drain_inst.ins, ScopedClock({None: tick_clock.global_clock})
            )
            popped = self.nc._tile_sem_poison_stack.pop()
            assert popped is self._sem_poison

    f32 = mybir.dt.float32
    AF = mybir.ActivationFunctionType

    nc = bacc.Bacc("TRN2")

    split = precision == "split"
    # matmul operand dtype: bf16 for bf16/split schemes, f32r (full fp32
    # storage, reduced-precision PE matmul at bf16 speed) for "f32r"
    bf = mybir.dt.float32r if precision == "f32r" else mybir.dt.bfloat16
    bfsq = mybir.dt.bfloat16  # sumsq path: bf16 is plenty (var is benign)
    map_passes = [("hi", "hi"), ("hi", "lo"), ("lo", "hi")] if split else [("", "")]
    sfx = ("hi", "lo") if split else ("",)

    xt = {s: nc.dram_tensor(f"xt{s}", [128, 2, 4, 512], bf, kind="ExternalInput")
          for s in sfx}
    wm = {s: nc.dram_tensor(f"wm{s}", [128, 8, 4, 128], bf, kind="ExternalInput")
          for s in sfx}
    va = {s: nc.dram_tensor(f"va{s}", [128, 8, NAUG], bf, kind="ExternalInput")
          for s in sfx}
    # out[:, 0, :] = Y^T (32 scores + colsum + bcross); out[0, 1, :] = sumsq
    out_h = nc.dram_tensor("out", [NAUG, 2, JPC], f32, kind="ExternalOutput")

    NJT = JPC // 512  # 2 j-tiles

    with LeanTailTileContext(nc) as tc:
        with (
            tc.tile_pool(name="cst", bufs=1) as cst,
            tc.tile_pool(name="big", bufs=1) as big,
            tc.tile_pool(name="ps_map", bufs=4, space="PSUM") as ps_map,
            tc.tile_pool(name="ps_sml", bufs=2, space="PSUM") as ps_sml,
        ):
            xt_sb = {s: big.tile([128, 2, 4, 512], bf, name=f"xt{s}_sb", tag=f"xt{s}") for s in sfx}
            wm_sb = {s: big.tile([128, 8, 4, 128], bf, name=f"wm{s}_sb", tag=f"wm{s}") for s in sfx}
            va_sb = {s: cst.tile([128, 8, NAUG], bf, name=f"va{s}_sb", tag=f"va{s}") for s in sfx}
            tb_sb = {s: big.tile([128, 8, JPC], bf, name=f"tb{s}_sb", tag=f"tb{s}") for s in sfx}
            sq_sb = {s: big.tile([128, 8, JPC], bfsq, name=f"sq{s}_sb", tag=f"sq{s}") for s in sfx}
            mo_sb_f = cst.tile([128, 1], bfsq, name="mo_sb")  # ones column
            out_sb = cst.tile([NAUG, 2, JPC], f32)

            # Few DMA instructions (each costs ~650ns of sequencer issue
            # time), critical-path data first, all on one ring so the
            # early transfers aren't bandwidth-shared with the bulk.
            for s in sfx:
                nc.sync.dma_start(wm_sb[s][:, 0:2], wm[s][:, 0:2])
                nc.sync.dma_start(xt_sb[s][:, 0], xt[s][:, 0])
            for s in sfx:
                nc.sync.dma_start(wm_sb[s][:, 2:8], wm[s][:, 2:8])
            for s in sfx:
                nc.sync.dma_start(xt_sb[s][:, 1], xt[s][:, 1])
                nc.sync.dma_start(va_sb[s][:], va[s][:])
            nc.vector.memset(mo_sb_f[:], 1.0)
            nc.vector.memset(out_sb[:, 1, :], 0.0)
            mo_sb = mo_sb_f

            # PE warm-up during the DMA fill: dependency-free matmuls keep
            # the HAM activity monitor busy so the real matmuls start at
            # 2.4 GHz instead of the 1.2 GHz cold clock.
            if warmup:
                warm_sb = cst.tile([128, 256], mybir.dt.bfloat16,
                                   name="warm_sb")
                nc.vector.memset(warm_sb[:], 0.25)
                psw = ps_map.tile([128, 512], f32, name="psmap", tag="psmap")
                for w in range(warmup):
                    nc.tensor.matmul(
                        psw[:, 0:256], warm_sb[:, 0:128], warm_sb[:],
                        start=True, stop=True,
                    )

            # ---- token projection: T^T[d, j] = sum_i W[i,d] * xT[i,j] ----
            for jt in range(NJT):
                for dc in range(8):
                    ps = ps_map.tile([128, 512], f32, name="psmap", tag="psmap")
                    nmm = len(map_passes) * 4
                    k = 0
                    for (ls, rs) in map_passes:
                        for i in range(4):
                            nc.tensor.matmul(
                                ps[:],
                                wm_sb[ls][:, dc, i, :],
                                xt_sb[rs][:, jt, i, :],
                                start=(k == 0),
                                stop=(k == nmm - 1),
                            )
                            k += 1
                    jsl = slice(jt * 512, (jt + 1) * 512)
                    if split:
                        nc.vector.tensor_copy(tb_sb["hi"][:, dc, jsl], ps[:])
                        nc.vector.tensor_sub(
                            tb_sb["lo"][:, dc, jsl], ps[:], tb_sb["hi"][:, dc, jsl]
                        )
                        sqf = big.tile([128, 512], f32, name="sqf",
                                       tag="sqf", bufs=16)
                        nc.scalar.activation(sqf[:], ps[:], AF.Square)
                        nc.scalar.activation(
                            sq_sb["hi"][:, dc, jsl], sqf[:], AF.Copy
                        )
                        nc.vector.tensor_sub(
                            sq_sb["lo"][:, dc, jsl], sqf[:],
                            sq_sb["hi"][:, dc, jsl],
                        )
                    else:
                        nc.vector.tensor_copy(tb_sb[""][:, dc, jsl], ps[:])
                        nc.scalar.activation(
                            sq_sb[""][:, dc, jsl], ps[:], AF.Square
                        )

            # ---- scores/stats via Vaug^T @ T^T;  sumsq via ones^T @ SQ ----
            if split:
                sc_passes = [("hi", "hi"), ("hi", "lo"), ("lo", "hi")]
                sq_passes = [("", "hi"), ("", "lo")]
            else:
                sc_passes = [("", "")]
                sq_passes = [("", "")]
            for jt in range(NJT):
                jsl = slice(jt * 512, (jt + 1) * 512)
                psy = ps_sml.tile([NAUG, 512], f32, name="psy", tag="psy")
                nmm = len(sc_passes) * 8
                k = 0
                for (ls, rs) in sc_passes:
                    for dc in range(8):
                        nc.tensor.matmul(
                            psy[:],
                            va_sb[ls][:, dc, :],
                            tb_sb[rs][:, dc, jsl],
                            start=(k == 0),
                            stop=(k == nmm - 1),
                        )
                        k += 1
                nc.vector.tensor_copy(out_sb[0:NAUG, 0, jsl], psy[:])
                # ship the Y half now; the sumsq half follows while this
                # transfer drains (2D APs, one region per DMA)
                nc.sync.dma_start(out_h[:, 0, jsl], out_sb[:, 0, jsl])

                pss = ps_sml.tile([1, 512], f32, name="pss", tag="pss")
                nmm = len(sq_passes) * 8
                k = 0
                for (_, rs) in sq_passes:
                    for dc in range(8):
                        nc.tensor.matmul(
                            pss[:],
                            mo_sb[:],
                            sq_sb[rs][:, dc, jsl],
                            start=(k == 0),
                            stop=(k == nmm - 1),
                        )
                        k += 1
                nc.vector.tensor_copy(out_sb[0:1, 1, jsl], pss[:])
                nc.sync.dma_start(out_h[0:1, 1, jsl], out_sb[0:1, 1, jsl])

    nc.compile()
    return nc


def _host_precompute(inputs):
    x = np.asarray(inputs["x"], np.float32)
    W = np.asarray(inputs["W_map"], np.float32)
    b_map = np.asarray(inputs["b_map"], np.float32)
    g = np.asarray(inputs["ln_g"], np.float32)
    lb = np.asarray(inputs["ln_b"], np.float32)
    Wq = np.asarray(inputs["Wq"], np.float32)
    bq = np.asarray(inputs["bq"], np.float32)
    Wk = np.asarray(inputs["Wk"], np.float32)
    bk = np.asarray(inputs["bk"], np.float32)
    tt = np.asarray(inputs["time_token"], np.float32)
    ct = np.asarray(inputs["cluster_token"], np.float32)

    spec = np.concatenate([tt, ct], 0)                      # [2, D]
    mu = spec.mean(-1, keepdims=True)
    var = ((spec - mu) ** 2).mean(-1, keepdims=True)
    hspec = ((spec - mu) / np.sqrt(var + EPS) * g + lb).reshape(2, H, DH)
    q = np.einsum("rhd,hde->rhe", hspec, Wq) + bq[None]
    qs = (q / np.sqrt(DH)).astype(np.float32)               # [2,H,DH]
    kspec = np.einsum("rhd,hde->rhe", hspec, Wk) + bk[None]
    s_spec = np.einsum("rhe,the->hrt", qs, kspec)           # [H,2,2]

    v = np.einsum("hde,rhe->hdr", Wk, qs)                   # [H,DH,2]
    V = np.zeros((D, 2 * H), np.float32)
    for h in range(H):
        V[64 * h:64 * h + 64, 2 * h] = v[h, :, 0]
        V[64 * h:64 * h + 64, 2 * h + 1] = v[h, :, 1]
    c0 = np.empty(2 * H, np.float32)
    for h in range(H):
        c0[2 * h] = qs[0, h] @ bk[h]
        c0[2 * h + 1] = qs[1, h] @ bk[h]

    Vg = g[:, None] * V
    # augmented score matrix: [Vg | ones | b_map]
    Vaug = np.concatenate(
        [Vg, np.ones((D, 1), np.float32), b_map[:, None]], 1)  # [D, 34]
    consts = dict(
        pg=Vg.sum(0),
        qb=(lb[:, None] * V).sum(0),
        bVg=(b_map[:, None] * Vg).sum(0),
        bmean=b_map.mean(),
        bsq=(b_map ** 2).sum(),
        s_spec=s_spec,
        c0=c0,
    )
    return x, Vaug, W, consts


def kernel(**inputs):
    from concourse.bass_utils import run_bass_kernel_spmd

    x, Vaug, W, consts = _host_precompute(inputs)

    key = (PRECISION, WARMUP_MMS)
    if key not in _PROG_CACHE:
        _PROG_CACHE[key] = _build_program(PRECISION, WARMUP_MMS)
    nc = _PROG_CACHE[key]

    split = PRECISION == "split"

    xf = x.reshape(B * N, IN_D)
    xpad = np.zeros((JTOT, IN_D), np.float32)
    xpad[:B * N] = xf

    def pmaj(a, k):
        # [k*128, n] -> partition-major [128, k, n]
        return np.ascontiguousarray(
            np.asarray(a).reshape(k, 128, -1).transpose(1, 0, 2))

    def pack_wm(a):
        # [512, 1024] -> [128p, 8dc, 4i, 128]
        return np.ascontiguousarray(
            np.asarray(a).reshape(4, 128, 8, 128).transpose(1, 2, 0, 3))

    def pack_xt(a):
        # [512, 1024] -> [128p, 2jt, 4i, 512]
        return np.ascontiguousarray(
            np.asarray(a).reshape(4, 128, 2, 512).transpose(1, 2, 0, 3))

    shared = {}
    if split:
        Whi, Wlo = _split_hi_lo(W)
        Vhi, Vlo = _split_hi_lo(Vaug)
        shared["wmhi"] = pack_wm(Whi)
        shared["wmlo"] = pack_wm(Wlo)
        shared["vahi"] = pmaj(Vhi, 8)
        shared["valo"] = pmaj(Vlo, 8)
    elif PRECISION == "f32r":
        shared["wm"] = pack_wm(W)
        shared["va"] = pmaj(Vaug, 8)
    else:
        shared["wm"] = pack_wm(_bf16(W))
        shared["va"] = pmaj(_bf16(Vaug), 8)

    in_maps = []
    for c in range(NCORES):
        xT = np.ascontiguousarray(xpad[c * JPC:(c + 1) * JPC].T)  # [512, 1024]
        m = dict(shared)
        if split:
            xh, xl = _split_hi_lo(xT)
            m["xthi"] = pack_xt(xh)
            m["xtlo"] = pack_xt(xl)
        elif PRECISION == "f32r":
            m["xt"] = pack_xt(xT)
        else:
            m["xt"] = pack_xt(_bf16(xT))
        in_maps.append(m)

    trace = bool(int(os.environ.get("AS_TRACE", "0")))
    res = run_bass_kernel_spmd(nc, in_maps, list(range(NCORES)), trace=trace)
    global LAST_RESULT
    LAST_RESULT = res
    outs = [np.asarray(r["out"], np.float32) for r in res.results]

    return _epilogue(outs, consts)


def _epilogue(outs, consts):
    # outs: per-core [34, 2, JPC]; [:,0,:] = Y^T, [0,1,:] = sumsq
    yfull = np.concatenate([o[:, 0, :].T for o in outs], 0)[:B * N]
    SQ = np.concatenate([o[0, 1, :] for o in outs], 0)[:B * N]
    Y = yfull[:, 0:32]
    colsum = yfull[:, 32]
    bcross = yfull[:, 33]

    mu = colsum / np.float32(D) + consts["bmean"]
    E2 = (SQ + 2.0 * bcross + consts["bsq"]) / np.float32(D)
    var = E2 - mu ** 2
    rstd = (1.0 / np.sqrt(var + EPS)).astype(np.float32)
    G = Y + consts["bVg"][None]
    sc = (rstd[:, None] * G
          - (rstd * mu)[:, None] * consts["pg"][None]
          + consts["qb"][None] + consts["c0"][None])
    sc = sc.reshape(B, N, H, 2).transpose(0, 2, 3, 1)       # [B,H,2,N]

    scores = np.empty((B, H, 2, S), np.float32)
    scores[:, :, :, 2:] = sc
    scores[:, :, :, 0:2] = consts["s_spec"][None]

    m = scores - scores.max(-1, keepdims=True)
    e = np.exp(m)
    attn = e / e.sum(-1, keepdims=True)
    mm = attn.mean(1)                                       # [B,2,S]
    return (np.ascontiguousarray(mm[:, 0, :]),
            np.ascontiguousarray(mm[:, 1, :]))



# revision 26
# speedup vs baseline: 1.5178x; 1.5178x over previous
"""Trainium2 Bass kernel for nn_AttentionSiphon.

Reference computes: tokens = x @ W_map + b_map; concat [time, cluster, tokens];
LayerNorm; per-head q/k projections; softmax(q k^T / sqrt(dh)); mean over heads;
returns rows 0 and 1 of the [B,S,S] head-mean attention.

Only attention rows 0/1 are returned, and their queries come from the
(batch-independent) time/cluster tokens, so per-head attention collapses to
an affine function of each token's LN statistics:

  score[j, c] = rstd_j * (x_j . (W @ Vg)[:, c]) + affine(mu_j, rstd_j)

All linear-in-token quantities (32 score columns, the LN mean via colsum,
and the b_map cross term) collapse host-side into Wv = W_map @ Vaug, a
single [512, 34] matrix, so the device never materializes the [*,1024]
token projection.  The only nonlinear term is the LN variance's sum of
squares, a quadratic form x_j^T (W W^T) x_j = ||L^T x_j||^2 with
L = chol(W_map W_map^T) [512, 512] lower-triangular: 10 of 16 blocks.

The Z matmul is PIVOTED (Z^T = x^T L, psum partitions = tokens) so the
LN sum-of-squares reduces along the engines' free axis: the scalar
engine's activation(Square, accum_out=...) squares and row-sums a whole
[128, 512] psum group in one pass (exact f32, no ones-matmul, no SQ
materialization).  The triangular L rows stream as matmul rhs with
truncated widths (128/256/384/512), so the dead upper-triangle blocks
cost nothing.

Device per core (1024 rows, 8 j-groups of 128):
  Zt_g = x_g^T L     4 bf16 matmuls / group, n = 128..512
  SQ_g = rowsum(Zt_g^2)   ACT accum_out (6 groups) / DVE fused (2 groups)
  Y    = Wv^T x      4 bf16 matmuls / j-tile of 512 (scores+colsum+bcross)

Output per core: [34, 1024] f32 Y rows + [128, 8] f32 sumsq columns.
Softmax epilogue ([4,16,2,2048]) runs on host.
"""

import os
import sys

sys.path.insert(0, "/opt/trn_rl_repo")

import numpy as np
import ml_dtypes

B, N, IN_D = 4, 2046, 512
D, H, DH = 1024, 16, 64
S = N + 2
EPS = 1e-5
NCORES = 8
JPC = 1024            # padded rows per core
JTOT = NCORES * JPC   # 8192 (8184 real rows + 8 pad)
NAUG = 34             # 32 score cols + colsum + b_map cross
NJT = 2               # j-tiles of 512 per core

# Precision scheme: "bf16" (everything bf16, ~1.5e-3), "mixed" (L rows 2,3
# + x chunks 2,3 in fp8 via one DoubleRow matmul per group, ~4e-3)
PRECISION = os.environ.get("AS_PRECISION", "bf16")
WARMUP_MMS = int(os.environ.get("AS_WARM", "20"))
LOFF = [0, 128, 384, 768]     # tight-packed L row-block offsets (widths 128*(i+1))

_PROG_CACHE = {}
LAST_RESULT = None  # BassKernelResults of the most recent run (for test harness)


def _bf16(a):
    return np.asarray(a, np.float32).astype(ml_dtypes.bfloat16)


def _fp8(a):
    return np.asarray(a, np.float32).astype(ml_dtypes.float8_e4m3)


def _build_program(precision, warmup=None):
    if warmup is None:
        warmup = WARMUP_MMS
    import concourse.bacc as bacc
    import concourse.mybir as mybir
    from concourse import tile
    from concourse.tile import ScopedClock

    class LeanTailTileContext(tile.TileContext):
        """Skip the exit-path double all-engine barrier + per-sem clears.

        The kernel preamble (Bass.__init__, target_bir_lowering) already
        dma_reset+sem_clears the kernel sem range at the start of every
        execution, and this program has a single TileContext, so nothing
        downstream consumes the freed sems. The final Sync drain still
        waits on every proc (incl. DMA lanes), so outputs are complete
        before the instruction streams end.
        """

        def _drain_and_barrier(self, tick_clock, wait_clock):
            drain_inst = self.nc.sync.drain()
            wait_clock.add_sem_waits(
                drain_inst.ins, ScopedClock({None: tick_clock.global_clock})
            )
            popped = self.nc._tile_sem_poison_stack.pop()
            assert popped is self._sem_poison

    f32 = mybir.dt.float32
    bf = mybir.dt.bfloat16
    f8 = mybir.dt.float8e4
    AF = mybir.ActivationFunctionType
    DR = mybir.MatmulPerfMode.DoubleRow

    mixed = precision == "mixed"

    nc = bacc.Bacc("TRN2")

    # DRAM inputs (per core): x jt-major [128p, 2jt, 4i, 512j] so a j-tile's
    # DMA moves one contiguous 4KB line per partition; L rows tight-packed
    # [128p, 1280] (row-block i at LOFF[i], width 128*(i+1)); Wv [128p,4i,34]
    xb = nc.dram_tensor("xb", [128, 2, 4, 512], bf, kind="ExternalInput")
    lt = nc.dram_tensor("lt", [128, LOFF[3] + 512], bf, kind="ExternalInput")
    wv = nc.dram_tensor("wv", [128, 4, NAUG], bf, kind="ExternalInput")
    if mixed:
        # fp8 copies of L row-blocks 2,3 (full 512 width) + x chunks 2,3
        # for the one DoubleRow matmul per group
        lz = nc.dram_tensor("lz", [128, 2, 512], f8, kind="ExternalInput")
        xz = nc.dram_tensor("xz", [128, 2, 2, 512], f8, kind="ExternalInput")
    out_h = nc.dram_tensor("out", [NAUG, JPC], f32, kind="ExternalOutput")
    outs_h = nc.dram_tensor("outs", [128, 12], f32, kind="ExternalOutput")

    with LeanTailTileContext(nc) as tc:
        with (
            tc.tile_pool(name="cst", bufs=1) as cst,
            tc.tile_pool(name="big", bufs=1) as big,
            tc.tile_pool(name="ps_z", bufs=5, space="PSUM") as ps_z,
            tc.tile_pool(name="ps_y", bufs=2, space="PSUM") as ps_y,
        ):
            xb_sb = big.tile([128, 2, 4, 512], bf, name="xb_sb", tag="xb")
            lt_sb = cst.tile([128, LOFF[3] + 512], bf, name="lt_sb", tag="lt")
            wv_sb = cst.tile([128, 4, NAUG], bf, name="wv_sb", tag="wv")
            if mixed:
                lz_sb = cst.tile([128, 2, 512], f8, name="lz_sb", tag="lz")
                xz_sb = big.tile([128, 2, 2, 512], f8, name="xz_sb", tag="xz")
            # cols 0..7: ACT sum-of-squares per job col; cols 8:12: the two
            # DVE bn_stats jobs' (mean, var) pairs -> host: 512*(var+mean^2)
            out_sb = cst.tile([NAUG, JPC], f32)
            sq_sb = cst.tile([128, 12], f32, name="sq_sb")
            zsc_a = big.tile([128, 512], bf, name="zsc_a")   # ACT scratch
            st_v = big.tile([128, 6], f32, name="st_v")      # bn_stats scratch

            nc.gpsimd.memset(sq_sb[:], 0.0)
            # critical-path data first: L + x jt0 feed the first Z groups.
            nc.sync.dma_start(lt_sb[:], lt[:])
            if mixed:
                nc.sync.dma_start(lz_sb[:], lz[:])
                nc.sync.dma_start(xz_sb[:, 0], xz[:, 0])
            nc.sync.dma_start(xb_sb[:, 0], xb[:, 0])
            nc.sync.dma_start(wv_sb[:], wv[:])
            if mixed:
                nc.sync.dma_start(xz_sb[:, 1], xz[:, 1])
            nc.sync.dma_start(xb_sb[:, 1], xb[:, 1])

            # PE warm-up during the DMA fill: dependency-free matmuls keep
            # the HAM activity monitor busy so the real matmuls start at
            # 2.4 GHz instead of the 1.2 GHz cold clock.
            if warmup:
                warm_sb = cst.tile([128, 256], bf, name="warm_sb")
                nc.vector.memset(warm_sb[:], 0.25)
                psw = ps_z.tile([128, 512], f32, name="psz", tag="psz")
                for w in range(warmup):
                    nc.tensor.matmul(
                        psw[:, 0:256], warm_sb[:, 0:128], warm_sb[:],
                        start=True, stop=True,
                    )

            # square-reduce job engine per column (jt*4+g): DVE-fused for
            # two mid-stream groups, ACT accum_out for the rest
            dve_cols = {2, 5}
            for jt in range(NJT):
                jsl = slice(jt * 512, (jt + 1) * 512)
                for g in range(4):
                    jb = slice(g * 128, g * 128 + 128)
                    col = jt * 4 + g
                    psz = ps_z.tile([128, 512], f32, name="psz", tag="psz")
                    # widest matmul first so start=True covers the full
                    # [128,512] psum region before narrower accumulates
                    if mixed:
                        nc.tensor.matmul(
                            psz[:], xz_sb[:, jt, :, jb], lz_sb[:],
                            start=True, stop=False, perf_mode=DR,
                        )
                        order = [1, 0]
                    else:
                        nc.tensor.matmul(
                            psz[:], xb_sb[:, jt, 3, jb],
                            lt_sb[:, LOFF[3]:LOFF[3] + 512],
                            start=True, stop=False,
                        )
                        order = [2, 1, 0]
                    for k, i in enumerate(order):
                        n = 128 * (i + 1)
                        nc.tensor.matmul(
                            psz[:, 0:n], xb_sb[:, jt, i, jb],
                            lt_sb[:, LOFF[i]:LOFF[i] + n],
                            start=False, stop=(k == len(order) - 1),
                        )
                    if col in dve_cols:
                        k = 8 + 2 * sorted(dve_cols).index(col)
                        nc.vector.bn_stats(st_v[:], psz[:])
                        nc.vector.bn_aggr(sq_sb[:, k:k + 2], st_v[:])
                    else:
                        nc.scalar.activation(
                            zsc_a[:], psz[:], AF.Square,
                            accum_out=sq_sb[:, col:col + 1],
                        )

                # ---- Y = Wv^T x (bf16), fills the PE while squares drain ----
                psy = ps_y.tile([NAUG, 512], f32, name="psy", tag="psy")
                for i in range(4):
                    nc.tensor.matmul(
                        psy[:], wv_sb[:, i, :], xb_sb[:, jt, i, :],
                        start=(i == 0), stop=(i == 3),
                    )
                nc.vector.tensor_copy(out_sb[:, jsl], psy[:])
                nc.sync.dma_start(out_h[:, jsl], out_sb[:, jsl])
            nc.sync.dma_start(outs_h[:], sq_sb[:])

    nc.compile()
    return nc


def _host_precompute(inputs):
    x = np.asarray(inputs["x"], np.float32)
    W = np.asarray(inputs["W_map"], np.float32)
    b_map = np.asarray(inputs["b_map"], np.float32)
    g = np.asarray(inputs["ln_g"], np.float32)
    lb = np.asarray(inputs["ln_b"], np.float32)
    Wq = np.asarray(inputs["Wq"], np.float32)
    bq = np.asarray(inputs["bq"], np.float32)
    Wk = np.asarray(inputs["Wk"], np.float32)
    bk = np.asarray(inputs["bk"], np.float32)
    tt = np.asarray(inputs["time_token"], np.float32)
    ct = np.asarray(inputs["cluster_token"], np.float32)

    spec = np.concatenate([tt, ct], 0)                      # [2, D]
    mu = spec.mean(-1, keepdims=True)
    var = ((spec - mu) ** 2).mean(-1, keepdims=True)
    hspec = ((spec - mu) / np.sqrt(var + EPS) * g + lb).reshape(2, H, DH)
    q = np.einsum("rhd,hde->rhe", hspec, Wq) + bq[None]
    qs = (q / np.sqrt(DH)).astype(np.float32)               # [2,H,DH]
    kspec = np.einsum("rhd,hde->rhe", hspec, Wk) + bk[None]
    s_spec = np.einsum("rhe,the->hrt", qs, kspec)           # [H,2,2]

    v = np.einsum("hde,rhe->hdr", Wk, qs)                   # [H,DH,2]
    V = np.zeros((D, 2 * H), np.float32)
    for h in range(H):
        V[64 * h:64 * h + 64, 2 * h] = v[h, :, 0]
        V[64 * h:64 * h + 64, 2 * h + 1] = v[h, :, 1]
    c0 = np.empty(2 * H, np.float32)
    for h in range(H):
        c0[2 * h] = qs[0, h] @ bk[h]
        c0[2 * h + 1] = qs[1, h] @ bk[h]

    Vg = g[:, None] * V
    # augmented score matrix: [Vg | ones | b_map]
    Vaug = np.concatenate(
        [Vg, np.ones((D, 1), np.float32), b_map[:, None]], 1)  # [D, 34]
    consts = dict(
        pg=Vg.sum(0),
        qb=(lb[:, None] * V).sum(0),
        bVg=(b_map[:, None] * Vg).sum(0),
        bmean=b_map.mean(),
        bsq=(b_map ** 2).sum(),
        s_spec=s_spec,
        c0=c0,
    )

    Wd = W.astype(np.float64)
    Wv = (Wd @ Vaug.astype(np.float64)).astype(np.float32)   # [512, 34]
    G = Wd @ Wd.T
    L = np.linalg.cholesky(G).astype(np.float32)             # [512, 512] lower
    return x, Wv, L, consts


def _pack_L(L):
    """[512, 512] lower-tri -> tight rhs pack [128p, 1280] bf16: row-block i
    (L[128i:128i+128, :]) at column offset LOFF[i], width 128*(i+1)."""
    Lb = _bf16(L)
    out = np.zeros((128, LOFF[3] + 512), ml_dtypes.bfloat16)
    for i in range(4):
        n = 128 * (i + 1)
        out[:, LOFF[i]:LOFF[i] + n] = Lb[128 * i:128 * i + 128, 0:n]
    return out


def kernel(**inputs):
    from concourse.bass_utils import run_bass_kernel_spmd

    x, Wv, L, consts = _host_precompute(inputs)

    key = (PRECISION, WARMUP_MMS)
    if key not in _PROG_CACHE:
        _PROG_CACHE[key] = _build_program(PRECISION, WARMUP_MMS)
    nc = _PROG_CACHE[key]

    mixed = PRECISION == "mixed"

    xf = x.reshape(B * N, IN_D)
    xpad = np.zeros((JTOT, IN_D), np.float32)
    xpad[:B * N] = xf

    def pack_x(a, dt):
        # [512, 1024] -> [128p, 2jt, 4i, 512j]
        return np.ascontiguousarray(
            np.asarray(a).astype(dt).reshape(4, 128, 2, 512)
            .transpose(1, 2, 0, 3))

    shared = {
        "lt": _pack_L(L),
        "wv": np.ascontiguousarray(
            _bf16(Wv).reshape(4, 128, NAUG).transpose(1, 0, 2)),
    }
    if mixed:
        # fp8 L row-blocks 2,3 at full 512 width for the DoubleRow matmul
        L8 = _fp8(L)
        shared["lz"] = np.ascontiguousarray(
            np.stack([L8[256:384, :], L8[384:512, :]], 1))

    in_maps = []
    for c in range(NCORES):
        xT = np.ascontiguousarray(xpad[c * JPC:(c + 1) * JPC].T)  # [512, 1024]
        m = dict(shared)
        m["xb"] = pack_x(xT, ml_dtypes.bfloat16)
        if mixed:
            m["xz"] = np.ascontiguousarray(
                pack_x(xT, ml_dtypes.float8_e4m3)[:, :, 2:4, :])
        in_maps.append(m)

    trace = bool(int(os.environ.get("AS_TRACE", "0")))
    res = run_bass_kernel_spmd(nc, in_maps, list(range(NCORES)), trace=trace)
    global LAST_RESULT
    LAST_RESULT = res
    outs = [(np.asarray(r["out"], np.float32), np.asarray(r["outs"], np.float32))
            for r in res.results]

    return _epilogue(outs, consts)


DVE_COLS = (2, 5)


def _sq_from_outs(s):
    # s: [128, 12]: cols 0:8 ACT sum-of-squares; cols 8:12 the DVE jobs'
    # bn_stats (mean, var) pairs -> SQ = 512*(var + mean^2)
    sq = np.array(s[:, 0:8])
    for k, col in enumerate(DVE_COLS):
        mean = s[:, 8 + 2 * k]
        var = s[:, 9 + 2 * k]
        sq[:, col] = 512.0 * (var + mean * mean)
    return sq.T.reshape(JPC)


def _epilogue(outs, consts):
    # outs: per-core ([34, JPC] Y^T, [128, 12] sumsq data: SQ[128g+p])
    yfull = np.concatenate([o[0].T for o in outs], 0)[:B * N]
    SQ = np.concatenate([_sq_from_outs(o[1]) for o in outs], 0)[:B * N]
    Y = yfull[:, 0:32]
    colsum = yfull[:, 32]
    bcross = yfull[:, 33]

    mu = colsum / np.float32(D) + consts["bmean"]
    E2 = (SQ + 2.0 * bcross + consts["bsq"]) / np.float32(D)
    var = E2 - mu ** 2
    rstd = (1.0 / np.sqrt(var + EPS)).astype(np.float32)
    G = Y + consts["bVg"][None]
    sc = (rstd[:, None] * G
          - (rstd * mu)[:, None] * consts["pg"][None]
          + consts["qb"][None] + consts["c0"][None])
    sc = sc.reshape(B, N, H, 2).transpose(0, 2, 3, 1)       # [B,H,2,N]

    scores = np.empty((B, H, 2, S), np.float32)
    scores[:, :, :, 2:] = sc
    scores[:, :, :, 0:2] = consts["s_spec"][None]

    m = scores - scores.max(-1, keepdims=True)
    e = np.exp(m)
    attn = e / e.sum(-1, keepdims=True)
    mm = attn.mean(1)                                       # [B,2,S]
    return (np.ascontiguousarray(mm[:, 0, :]),
            np.ascontiguousarray(mm[:, 1, :]))


# revision 28
# speedup vs baseline: 1.6655x; 1.0973x over previous
"""Trainium2 Bass kernel for nn_AttentionSiphon.

Reference computes: tokens = x @ W_map + b_map; concat [time, cluster, tokens];
LayerNorm; per-head q/k projections; softmax(q k^T / sqrt(dh)); mean over heads;
returns rows 0 and 1 of the [B,S,S] head-mean attention.

Only attention rows 0/1 are returned, and their queries come from the
(batch-independent) time/cluster tokens, so per-head attention collapses to
an affine function of each token's LN statistics:

  score[j, c] = rstd_j * (x_j . (W @ Vg)[:, c]) + affine(mu_j, rstd_j)

All linear-in-token quantities (32 score columns, the LN mean via colsum,
and the b_map cross term) collapse host-side into Wv = W_map @ Vaug, a
single [512, 34] matrix, so the device never materializes the [*,1024]
token projection.  The only nonlinear term is the LN variance's sum of
squares, a quadratic form x_j^T (W W^T) x_j = ||L^T x_j||^2 with
L = chol(W_map W_map^T) [512, 512] lower-triangular: 10 of 16 blocks.

The Z matmul is PIVOTED (Z^T = x^T L, psum partitions = tokens) so the
LN sum-of-squares reduces along the engines' free axis: the scalar
engine's activation(Square, accum_out=...) squares and row-sums a whole
[128, 512] psum group in one pass (exact f32, no ones-matmul, no SQ
materialization).  The triangular L rows stream as matmul rhs with
truncated widths (128/256/384/512), so the dead upper-triangle blocks
cost nothing.

Device per core (1024 rows, 8 j-groups of 128):
  Zt_g = x_g^T L     4 bf16 matmuls / group, n = 128..512
  SQ_g = rowsum(Zt_g^2)   ACT accum_out (6 groups) / DVE fused (2 groups)
  Y    = Wv^T x      4 bf16 matmuls / j-tile of 512 (scores+colsum+bcross)

Output per core: [34, 1024] f32 Y rows + [128, 8] f32 sumsq columns.
Softmax epilogue ([4,16,2,2048]) runs on host.
"""

import os
import sys

sys.path.insert(0, "/opt/trn_rl_repo")

import numpy as np
import ml_dtypes

B, N, IN_D = 4, 2046, 512
D, H, DH = 1024, 16, 64
S = N + 2
EPS = 1e-5
NCORES = 8
JPC = 1024            # padded rows per core
JTOT = NCORES * JPC   # 8192 (8184 real rows + 8 pad)
NAUG = 34             # 32 score cols + colsum + b_map cross
NJT = 2               # j-tiles of 512 per core

# Precision scheme: "bf16" (everything bf16, ~1.5e-3), "mixed" (L rows 2,3
# + x chunks 2,3 in fp8 via one DoubleRow matmul per group, ~4e-3)
PRECISION = os.environ.get("AS_PRECISION", "bf16")
WARMUP_MMS = int(os.environ.get("AS_WARM", "20"))
LOFF = [0, 128, 384, 768]     # tight-packed L row-block offsets (widths 128*(i+1))

_PROG_CACHE = {}
LAST_RESULT = None  # BassKernelResults of the most recent run (for test harness)


def _bf16(a):
    return np.asarray(a, np.float32).astype(ml_dtypes.bfloat16)


def _fp8(a):
    return np.asarray(a, np.float32).astype(ml_dtypes.float8_e4m3)


def _build_program(precision, warmup=None):
    if warmup is None:
        warmup = WARMUP_MMS
    import concourse.bacc as bacc
    import concourse.mybir as mybir
    from concourse import tile
    from concourse.tile import ScopedClock

    class LeanTailTileContext(tile.TileContext):
        """Skip the exit-path double all-engine barrier + per-sem clears.

        The kernel preamble (Bass.__init__, target_bir_lowering) already
        dma_reset+sem_clears the kernel sem range at the start of every
        execution, and this program has a single TileContext, so nothing
        downstream consumes the freed sems. The final Sync drain still
        waits on every proc (incl. DMA lanes), so outputs are complete
        before the instruction streams end.
        """

        def _drain_and_barrier(self, tick_clock, wait_clock):
            drain_inst = self.nc.sync.drain()
            wait_clock.add_sem_waits(
                drain_inst.ins, ScopedClock({None: tick_clock.global_clock})
            )
            popped = self.nc._tile_sem_poison_stack.pop()
            assert popped is self._sem_poison

    f32 = mybir.dt.float32
    bf = mybir.dt.bfloat16
    f8 = mybir.dt.float8e4
    AF = mybir.ActivationFunctionType
    DR = mybir.MatmulPerfMode.DoubleRow

    mixed = precision == "mixed"

    nc = bacc.Bacc("TRN2")

    # DRAM inputs (per core): x jt-major [128p, 2jt, 4i, 512j] so a j-tile's
    # DMA moves one contiguous 4KB line per partition; L rows tight-packed
    # [128p, 1280] (row-block i at LOFF[i], width 128*(i+1)); Wv [128p,4i,34]
    xb = nc.dram_tensor("xb", [128, 2, 4, 512], bf, kind="ExternalInput")
    lt = nc.dram_tensor("lt", [128, LOFF[3] + 512], bf, kind="ExternalInput")
    wv = nc.dram_tensor("wv", [128, 4, NAUG], bf, kind="ExternalInput")
    if mixed:
        # fp8 copies of L row-blocks 2,3 (full 512 width) + x chunks 2,3
        # for the one DoubleRow matmul per group
        lz = nc.dram_tensor("lz", [128, 2, 512], f8, kind="ExternalInput")
        xz = nc.dram_tensor("xz", [128, 2, 2, 512], f8, kind="ExternalInput")
    out_h = nc.dram_tensor("out", [NAUG, JPC], f32, kind="ExternalOutput")
    outs_h = nc.dram_tensor("outs", [128, 12], f32, kind="ExternalOutput")

    with LeanTailTileContext(nc) as tc:
        with (
            tc.tile_pool(name="cst", bufs=1) as cst,
            tc.tile_pool(name="big", bufs=1) as big,
            tc.tile_pool(name="ps_z", bufs=5, space="PSUM") as ps_z,
            tc.tile_pool(name="ps_y", bufs=2, space="PSUM") as ps_y,
        ):
            xb_sb = big.tile([128, 2, 4, 512], bf, name="xb_sb", tag="xb")
            lt_sb = cst.tile([128, LOFF[3] + 512], bf, name="lt_sb", tag="lt")
            wv_sb = cst.tile([128, 4, NAUG], bf, name="wv_sb", tag="wv")
            if mixed:
                lz_sb = cst.tile([128, 2, 512], f8, name="lz_sb", tag="lz")
                xz_sb = big.tile([128, 2, 2, 512], f8, name="xz_sb", tag="xz")
            # cols 0..7: ACT sum-of-squares per job col; cols 8:12: the two
            # DVE bn_stats jobs' (mean, var) pairs -> host: 512*(var+mean^2)
            out_sb = cst.tile([NAUG, JPC], f32)
            sq_sb = cst.tile([128, 12], f32, name="sq_sb")
            zsc_a = big.tile([128, 512], bf, name="zsc_a")   # ACT scratch
            st_v = big.tile([128, 6], f32, name="st_v")      # bn_stats scratch

            nc.gpsimd.memset(sq_sb[:], 0.0)
            # critical-path data first: the opening Z matmuls go widest-first
            # (i=3,2,...), so L rows 2-3 + x chunks 2-3 of jt0 unblock the PE
            # soonest; the rest follows while the first groups run.
            nc.sync.dma_start(lt_sb[:, LOFF[2]:], lt[:, LOFF[2]:])
            if mixed:
                nc.sync.dma_start(lz_sb[:], lz[:])
                nc.sync.dma_start(xz_sb[:, 0], xz[:, 0])
            nc.sync.dma_start(xb_sb[:, 0, 2:4], xb[:, 0, 2:4])
            nc.sync.dma_start(lt_sb[:, 0:LOFF[2]], lt[:, 0:LOFF[2]])
            nc.sync.dma_start(xb_sb[:, 0, 0:2], xb[:, 0, 0:2])
            nc.sync.dma_start(wv_sb[:], wv[:])
            if mixed:
                nc.sync.dma_start(xz_sb[:, 1], xz[:, 1])
            nc.sync.dma_start(xb_sb[:, 1], xb[:, 1])

            # PE warm-up during the DMA fill: dependency-free matmuls keep
            # the HAM activity monitor busy so the real matmuls start at
            # 2.4 GHz instead of the 1.2 GHz cold clock.
            if warmup:
                warm_sb = cst.tile([128, 256], bf, name="warm_sb")
                nc.vector.memset(warm_sb[:], 0.25)
                psw = ps_z.tile([128, 512], f32, name="psz", tag="psz")
                for w in range(warmup):
                    nc.tensor.matmul(
                        psw[:, 0:256], warm_sb[:, 0:128], warm_sb[:],
                        start=True, stop=True,
                    )

            # square-reduce job engine per column (jt*4+g): DVE bn_stats for
            # two mid-stream groups, ACT accum_out for the rest
            dve_cols = (2, 5)

            def z_group(jt, g):
                jb = slice(g * 128, g * 128 + 128)
                col = jt * 4 + g
                psz = ps_z.tile([128, 512], f32, name="psz", tag="psz")
                # widest matmul first so start=True covers the full
                # [128,512] psum region before narrower accumulates
                if mixed:
                    nc.tensor.matmul(
                        psz[:], xz_sb[:, jt, :, jb], lz_sb[:],
                        start=True, stop=False, perf_mode=DR,
                    )
                    order = [1, 0]
                else:
                    nc.tensor.matmul(
                        psz[:], xb_sb[:, jt, 3, jb],
                        lt_sb[:, LOFF[3]:LOFF[3] + 512],
                        start=True, stop=False,
                    )
                    order = [2, 1, 0]
                for k, i in enumerate(order):
                    n = 128 * (i + 1)
                    nc.tensor.matmul(
                        psz[:, 0:n], xb_sb[:, jt, i, jb],
                        lt_sb[:, LOFF[i]:LOFF[i] + n],
                        start=False, stop=(k == len(order) - 1),
                    )
                if col in dve_cols:
                    k = 8 + 2 * dve_cols.index(col)
                    nc.vector.bn_stats(st_v[:], psz[:])
                    nc.vector.bn_aggr(sq_sb[:, k:k + 2], st_v[:])
                else:
                    nc.scalar.activation(
                        zsc_a[:], psz[:], AF.Square,
                        accum_out=sq_sb[:, col:col + 1],
                    )

            def y_block(jt):
                jsl = slice(jt * 512, (jt + 1) * 512)
                psy = ps_y.tile([NAUG, 512], f32, name="psy", tag="psy")
                for i in range(4):
                    nc.tensor.matmul(
                        psy[:], wv_sb[:, i, :], xb_sb[:, jt, i, :],
                        start=(i == 0), stop=(i == 3),
                    )
                nc.vector.tensor_copy(out_sb[:, jsl], psy[:])
                nc.sync.dma_start(out_h[:, jsl], out_sb[:, jsl])

            for g in range(4):
                z_group(0, g)
            y_block(0)
            for g in range(3):
                z_group(1, g)
            # Y before the last Z group: its copy + DMA overlap the final
            # squares, shortening the tail
            y_block(1)
            z_group(1, 3)
            nc.scalar.dma_start(outs_h[:], sq_sb[:])

    nc.compile()
    return nc


def _host_precompute(inputs):
    x = np.asarray(inputs["x"], np.float32)
    W = np.asarray(inputs["W_map"], np.float32)
    b_map = np.asarray(inputs["b_map"], np.float32)
    g = np.asarray(inputs["ln_g"], np.float32)
    lb = np.asarray(inputs["ln_b"], np.float32)
    Wq = np.asarray(inputs["Wq"], np.float32)
    bq = np.asarray(inputs["bq"], np.float32)
    Wk = np.asarray(inputs["Wk"], np.float32)
    bk = np.asarray(inputs["bk"], np.float32)
    tt = np.asarray(inputs["time_token"], np.float32)
    ct = np.asarray(inputs["cluster_token"], np.float32)

    spec = np.concatenate([tt, ct], 0)                      # [2, D]
    mu = spec.mean(-1, keepdims=True)
    var = ((spec - mu) ** 2).mean(-1, keepdims=True)
    hspec = ((spec - mu) / np.sqrt(var + EPS) * g + lb).reshape(2, H, DH)
    q = np.einsum("rhd,hde->rhe", hspec, Wq) + bq[None]
    qs = (q / np.sqrt(DH)).astype(np.float32)               # [2,H,DH]
    kspec = np.einsum("rhd,hde->rhe", hspec, Wk) + bk[None]
    s_spec = np.einsum("rhe,the->hrt", qs, kspec)           # [H,2,2]

    v = np.einsum("hde,rhe->hdr", Wk, qs)                   # [H,DH,2]
    V = np.zeros((D, 2 * H), np.float32)
    for h in range(H):
        V[64 * h:64 * h + 64, 2 * h] = v[h, :, 0]
        V[64 * h:64 * h + 64, 2 * h + 1] = v[h, :, 1]
    c0 = np.empty(2 * H, np.float32)
    for h in range(H):
        c0[2 * h] = qs[0, h] @ bk[h]
        c0[2 * h + 1] = qs[1, h] @ bk[h]

    Vg = g[:, None] * V
    # augmented score matrix: [Vg | ones | b_map]
    Vaug = np.concatenate(
        [Vg, np.ones((D, 1), np.float32), b_map[:, None]], 1)  # [D, 34]
    consts = dict(
        pg=Vg.sum(0),
        qb=(lb[:, None] * V).sum(0),
        bVg=(b_map[:, None] * Vg).sum(0),
        bmean=b_map.mean(),
        bsq=(b_map ** 2).sum(),
        s_spec=s_spec,
        c0=c0,
    )

    Wd = W.astype(np.float64)
    Wv = (Wd @ Vaug.astype(np.float64)).astype(np.float32)   # [512, 34]
    G = Wd @ Wd.T
    L = np.linalg.cholesky(G).astype(np.float32)             # [512, 512] lower
    return x, Wv, L, consts


def _pack_L(L):
    """[512, 512] lower-tri -> tight rhs pack [128p, 1280] bf16: row-block i
    (L[128i:128i+128, :]) at column offset LOFF[i], width 128*(i+1)."""
    Lb = _bf16(L)
    out = np.zeros((128, LOFF[3] + 512), ml_dtypes.bfloat16)
    for i in range(4):
        n = 128 * (i + 1)
        out[:, LOFF[i]:LOFF[i] + n] = Lb[128 * i:128 * i + 128, 0:n]
    return out


def kernel(**inputs):
    from concourse.bass_utils import run_bass_kernel_spmd

    x, Wv, L, consts = _host_precompute(inputs)

    key = (PRECISION, WARMUP_MMS)
    if key not in _PROG_CACHE:
        _PROG_CACHE[key] = _build_program(PRECISION, WARMUP_MMS)
    nc = _PROG_CACHE[key]

    mixed = PRECISION == "mixed"

    xf = x.reshape(B * N, IN_D)
    xpad = np.zeros((JTOT, IN_D), np.float32)
    xpad[:B * N] = xf

    def pack_x(a, dt):
        # [512, 1024] -> [128p, 2jt, 4i, 512j]
        return np.ascontiguousarray(
            np.asarray(a).astype(dt).reshape(4, 128, 2, 512)
            .transpose(1, 2, 0, 3))

    shared = {
        "lt": _pack_L(L),
        "wv": np.ascontiguousarray(
            _bf16(Wv).reshape(4, 128, NAUG).transpose(1, 0, 2)),
    }
    if mixed:
        # fp8 L row-blocks 2,3 at full 512 width for the DoubleRow matmul
        L8 = _fp8(L)
        shared["lz"] = np.ascontiguousarray(
            np.stack([L8[256:384, :], L8[384:512, :]], 1))

    in_maps = []
    for c in range(NCORES):
        xT = np.ascontiguousarray(xpad[c * JPC:(c + 1) * JPC].T)  # [512, 1024]
        m = dict(shared)
        m["xb"] = pack_x(xT, ml_dtypes.bfloat16)
        if mixed:
            m["xz"] = np.ascontiguousarray(
                pack_x(xT, ml_dtypes.float8_e4m3)[:, :, 2:4, :])
        in_maps.append(m)

    trace = bool(int(os.environ.get("AS_TRACE", "0")))
    res = run_bass_kernel_spmd(nc, in_maps, list(range(NCORES)), trace=trace)
    global LAST_RESULT
    LAST_RESULT = res
    outs = [(np.asarray(r["out"], np.float32), np.asarray(r["outs"], np.float32))
            for r in res.results]

    return _epilogue(outs, consts)


DVE_COLS = (2, 5)


def _sq_from_outs(s):
    # s: [128, 12]: cols 0:8 ACT sum-of-squares; cols 8:12 the DVE jobs'
    # bn_stats (mean, var) pairs -> SQ = 512*(var + mean^2)
    sq = np.array(s[:, 0:8])
    for k, col in enumerate(DVE_COLS):
        mean = s[:, 8 + 2 * k]
        var = s[:, 9 + 2 * k]
        sq[:, col] = 512.0 * (var + mean * mean)
    return sq.T.reshape(JPC)


def _epilogue(outs, consts):
    # outs: per-core ([34, JPC] Y^T, [128, 12] sumsq data: SQ[128g+p])
    yfull = np.concatenate([o[0].T for o in outs], 0)[:B * N]
    SQ = np.concatenate([_sq_from_outs(o[1]) for o in outs], 0)[:B * N]
    Y = yfull[:, 0:32]
    colsum = yfull[:, 32]
    bcross = yfull[:, 33]

    mu = colsum / np.float32(D) + consts["bmean"]
    E2 = (SQ + 2.0 * bcross + consts["bsq"]) / np.float32(D)
    var = E2 - mu ** 2
    rstd = (1.0 / np.sqrt(var + EPS)).astype(np.float32)
    G = Y + consts["bVg"][None]
    sc = (rstd[:, None] * G
          - (rstd * mu)[:, None] * consts["pg"][None]
          + consts["qb"][None] + consts["c0"][None])
    sc = sc.reshape(B, N, H, 2).transpose(0, 2, 3, 1)       # [B,H,2,N]

    scores = np.empty((B, H, 2, S), np.float32)
    scores[:, :, :, 2:] = sc
    scores[:, :, :, 0:2] = consts["s_spec"][None]

    m = scores - scores.max(-1, keepdims=True)
    e = np.exp(m)
    attn = e / e.sum(-1, keepdims=True)
    mm = attn.mean(1)                                       # [B,2,S]
    return (np.ascontiguousarray(mm[:, 0, :]),
            np.ascontiguousarray(mm[:, 1, :]))


# revision 30
# speedup vs baseline: 1.7392x; 1.0442x over previous
"""Trainium2 Bass kernel for nn_AttentionSiphon.

Reference computes: tokens = x @ W_map + b_map; concat [time, cluster, tokens];
LayerNorm; per-head q/k projections; softmax(q k^T / sqrt(dh)); mean over heads;
returns rows 0 and 1 of the [B,S,S] head-mean attention.

Only attention rows 0/1 are returned, and their queries come from the
(batch-independent) time/cluster tokens, so per-head attention collapses to
an affine function of each token's LN statistics:

  score[j, c] = rstd_j * (x_j . (W @ Vg)[:, c]) + affine(mu_j, rstd_j)

All linear-in-token quantities (32 score columns, the LN mean via colsum,
and the b_map cross term) collapse host-side into Wv = W_map @ Vaug, a
single [512, 34] matrix, so the device never materializes the [*,1024]
token projection.  The only nonlinear term is the LN variance's sum of
squares, a quadratic form x_j^T (W W^T) x_j = ||L^T x_j||^2 with
L = chol(W_map W_map^T) [512, 512] lower-triangular: 10 of 16 blocks.

The Z matmul is PIVOTED (Z^T = x^T L, psum partitions = tokens) so the
LN sum-of-squares reduces along the engines' free axis: the scalar
engine's activation(Square, accum_out=...) squares and row-sums a whole
[128, 512] psum group in one pass (exact f32, no ones-matmul, no SQ
materialization).  The triangular L rows stream as matmul rhs with
truncated widths (128/256/384/512), so the dead upper-triangle blocks
cost nothing.

Device per core (1024 rows, 8 j-groups of 128):
  Zt_g = x_g^T L     4 bf16 matmuls / group, n = 128..512
  SQ_g = rowsum(Zt_g^2)   ACT accum_out (6 groups) / DVE fused (2 groups)
  Y    = Wv^T x      4 bf16 matmuls / j-tile of 512 (scores+colsum+bcross)

Output per core: [34, 1024] f32 Y rows + [128, 8] f32 sumsq columns.
Softmax epilogue ([4,16,2,2048]) runs on host.
"""

import os
import sys

sys.path.insert(0, "/opt/trn_rl_repo")

import numpy as np
import ml_dtypes

B, N, IN_D = 4, 2046, 512
D, H, DH = 1024, 16, 64
S = N + 2
EPS = 1e-5
NCORES = 8
JPC = 1024            # padded rows per core
JTOT = NCORES * JPC   # 8192 (8184 real rows + 8 pad)
NAUG = 34             # 32 score cols + colsum + b_map cross
NJT = 2               # j-tiles of 512 per core

# Precision scheme: "bf16" (everything bf16, ~1.5e-3), "mixed" (L rows 2,3
# + x chunks 2,3 in fp8 via one DoubleRow matmul per group, ~4e-3)
PRECISION = os.environ.get("AS_PRECISION", "bf16")
WARMUP_MMS = int(os.environ.get("AS_WARM", "14"))
LOFF = [0, 128, 384, 768]     # tight-packed L row-block offsets (widths 128*(i+1))

_PROG_CACHE = {}
LAST_RESULT = None  # BassKernelResults of the most recent run (for test harness)


def _bf16(a):
    return np.asarray(a, np.float32).astype(ml_dtypes.bfloat16)


def _fp8(a):
    return np.asarray(a, np.float32).astype(ml_dtypes.float8_e4m3)


def _build_program(precision, warmup=None):
    if warmup is None:
        warmup = WARMUP_MMS
    import concourse.bacc as bacc
    import concourse.mybir as mybir
    from concourse import tile
    from concourse.tile import ScopedClock

    class LeanTailTileContext(tile.TileContext):
        """Skip the exit-path double all-engine barrier + per-sem clears.

        The kernel preamble (Bass.__init__, target_bir_lowering) already
        dma_reset+sem_clears the kernel sem range at the start of every
        execution, and this program has a single TileContext, so nothing
        downstream consumes the freed sems. The final Sync drain still
        waits on every proc (incl. DMA lanes), so outputs are complete
        before the instruction streams end.
        """

        def _drain_and_barrier(self, tick_clock, wait_clock):
            drain_inst = self.nc.sync.drain()
            wait_clock.add_sem_waits(
                drain_inst.ins, ScopedClock({None: tick_clock.global_clock})
            )
            popped = self.nc._tile_sem_poison_stack.pop()
            assert popped is self._sem_poison

    f32 = mybir.dt.float32
    bf = mybir.dt.bfloat16
    f8 = mybir.dt.float8e4
    AF = mybir.ActivationFunctionType
    DR = mybir.MatmulPerfMode.DoubleRow

    mixed = precision == "mixed"

    nc = bacc.Bacc("TRN2")

    # DRAM inputs (per core): x jt-major [128p, 2jt, 4i, 512j] so a j-tile's
    # DMA moves one contiguous 4KB line per partition; L rows tight-packed
    # [128p, 1280] (row-block i at LOFF[i], width 128*(i+1)); Wv [128p,4i,34]
    xb = nc.dram_tensor("xb", [128, 2, 4, 512], bf, kind="ExternalInput")
    lt = nc.dram_tensor("lt", [128, LOFF[3] + 512], bf, kind="ExternalInput")
    wv = nc.dram_tensor("wv", [128, 4, NAUG], bf, kind="ExternalInput")
    if mixed:
        # fp8 copies of L row-blocks 2,3 (full 512 width) + x chunks 2,3
        # for the one DoubleRow matmul per group
        lz = nc.dram_tensor("lz", [128, 2, 512], f8, kind="ExternalInput")
        xz = nc.dram_tensor("xz", [128, 2, 2, 512], f8, kind="ExternalInput")
    out_h = nc.dram_tensor("out", [NAUG, JPC], f32, kind="ExternalOutput")
    outs_h = nc.dram_tensor("outs", [128, 12], f32, kind="ExternalOutput")

    with LeanTailTileContext(nc) as tc:
        with (
            tc.tile_pool(name="cst", bufs=1) as cst,
            tc.tile_pool(name="big", bufs=1) as big,
            tc.tile_pool(name="ps_z", bufs=5, space="PSUM") as ps_z,
            tc.tile_pool(name="ps_y", bufs=2, space="PSUM") as ps_y,
        ):
            xb_sb = big.tile([128, 2, 4, 512], bf, name="xb_sb", tag="xb")
            lt_sb = cst.tile([128, LOFF[3] + 512], bf, name="lt_sb", tag="lt")
            wv_sb = cst.tile([128, 4, NAUG], bf, name="wv_sb", tag="wv")
            if mixed:
                lz_sb = cst.tile([128, 2, 512], f8, name="lz_sb", tag="lz")
                xz_sb = big.tile([128, 2, 2, 512], f8, name="xz_sb", tag="xz")
            # cols 0..7: ACT sum-of-squares per job col; cols 8:12: the two
            # DVE bn_stats jobs' (mean, var) pairs -> host: 512*(var+mean^2)
            out_sb = cst.tile([NAUG, JPC], f32)
            sq_sb = cst.tile([128, 12], f32, name="sq_sb")
            zsc_a = big.tile([128, 512], bf, name="zsc_a")   # ACT scratch
            st_v = big.tile([128, 6], f32, name="st_v")      # bn_stats scratch

            nc.gpsimd.memset(sq_sb[:], 0.0)
            # critical-path data first, split across the two HWDGE rings
            # (SP + ACT issue queues run in parallel): the opening Z matmuls
            # go widest-first (i=3,2,...), so L rows 2-3 on one ring + x
            # chunks 2-3 of jt0 on the other unblock the PE soonest.
            nc.sync.dma_start(lt_sb[:, LOFF[2]:], lt[:, LOFF[2]:])
            nc.scalar.dma_start(xb_sb[:, 0, 2:4], xb[:, 0, 2:4])
            if mixed:
                nc.sync.dma_start(lz_sb[:], lz[:])
                nc.scalar.dma_start(xz_sb[:, 0], xz[:, 0])
            nc.sync.dma_start(lt_sb[:, 0:LOFF[2]], lt[:, 0:LOFF[2]])
            nc.scalar.dma_start(xb_sb[:, 0, 0:2], xb[:, 0, 0:2])
            nc.sync.dma_start(wv_sb[:], wv[:])
            if mixed:
                nc.sync.dma_start(xz_sb[:, 1], xz[:, 1])
            nc.scalar.dma_start(xb_sb[:, 1, 2:4], xb[:, 1, 2:4])
            nc.sync.dma_start(xb_sb[:, 1, 0:2], xb[:, 1, 0:2])

            # PE warm-up during the DMA fill: dependency-free matmuls keep
            # the HAM activity monitor busy so the real matmuls start at
            # 2.4 GHz instead of the 1.2 GHz cold clock.
            if warmup:
                warm_sb = cst.tile([128, 256], bf, name="warm_sb")
                nc.vector.memset(warm_sb[:], 0.25)
                psw = ps_z.tile([128, 512], f32, name="psz", tag="psz")
                for w in range(warmup):
                    nc.tensor.matmul(
                        psw[:, 0:256], warm_sb[:, 0:128], warm_sb[:],
                        start=True, stop=True,
                    )

            # square-reduce job engine per column (jt*4+g): DVE bn_stats for
            # two mid-stream groups, ACT accum_out for the rest
            dve_cols = (2, 5)

            def z_group(jt, g):
                jb = slice(g * 128, g * 128 + 128)
                col = jt * 4 + g
                psz = ps_z.tile([128, 512], f32, name="psz", tag="psz")
                # widest matmul first so start=True covers the full
                # [128,512] psum region before narrower accumulates
                if mixed:
                    nc.tensor.matmul(
                        psz[:], xz_sb[:, jt, :, jb], lz_sb[:],
                        start=True, stop=False, perf_mode=DR,
                    )
                    order = [1, 0]
                else:
                    nc.tensor.matmul(
                        psz[:], xb_sb[:, jt, 3, jb],
                        lt_sb[:, LOFF[3]:LOFF[3] + 512],
                        start=True, stop=False,
                    )
                    order = [2, 1, 0]
                for k, i in enumerate(order):
                    n = 128 * (i + 1)
                    nc.tensor.matmul(
                        psz[:, 0:n], xb_sb[:, jt, i, jb],
                        lt_sb[:, LOFF[i]:LOFF[i] + n],
                        start=False, stop=(k == len(order) - 1),
                    )
                if col in dve_cols:
                    k = 8 + 2 * dve_cols.index(col)
                    nc.vector.bn_stats(st_v[:], psz[:])
                    nc.vector.bn_aggr(sq_sb[:, k:k + 2], st_v[:])
                else:
                    nc.scalar.activation(
                        zsc_a[:], psz[:], AF.Square,
                        accum_out=sq_sb[:, col:col + 1],
                    )

            def y_block(jt):
                jsl = slice(jt * 512, (jt + 1) * 512)
                psy = ps_y.tile([NAUG, 512], f32, name="psy", tag="psy")
                for i in range(4):
                    nc.tensor.matmul(
                        psy[:], wv_sb[:, i, :], xb_sb[:, jt, i, :],
                        start=(i == 0), stop=(i == 3),
                    )
                nc.vector.tensor_copy(out_sb[:, jsl], psy[:])
                nc.sync.dma_start(out_h[:, jsl], out_sb[:, jsl])

            for g in range(4):
                z_group(0, g)
            y_block(0)
            for g in range(3):
                z_group(1, g)
            # Y before the last Z group: its copy + DMA overlap the final
            # squares, shortening the tail
            y_block(1)
            z_group(1, 3)
            nc.scalar.dma_start(outs_h[:], sq_sb[:])

    nc.compile()
    return nc


def _host_precompute(inputs):
    x = np.asarray(inputs["x"], np.float32)
    W = np.asarray(inputs["W_map"], np.float32)
    b_map = np.asarray(inputs["b_map"], np.float32)
    g = np.asarray(inputs["ln_g"], np.float32)
    lb = np.asarray(inputs["ln_b"], np.float32)
    Wq = np.asarray(inputs["Wq"], np.float32)
    bq = np.asarray(inputs["bq"], np.float32)
    Wk = np.asarray(inputs["Wk"], np.float32)
    bk = np.asarray(inputs["bk"], np.float32)
    tt = np.asarray(inputs["time_token"], np.float32)
    ct = np.asarray(inputs["cluster_token"], np.float32)

    spec = np.concatenate([tt, ct], 0)                      # [2, D]
    mu = spec.mean(-1, keepdims=True)
    var = ((spec - mu) ** 2).mean(-1, keepdims=True)
    hspec = ((spec - mu) / np.sqrt(var + EPS) * g + lb).reshape(2, H, DH)
    q = np.einsum("rhd,hde->rhe", hspec, Wq) + bq[None]
    qs = (q / np.sqrt(DH)).astype(np.float32)               # [2,H,DH]
    kspec = np.einsum("rhd,hde->rhe", hspec, Wk) + bk[None]
    s_spec = np.einsum("rhe,the->hrt", qs, kspec)           # [H,2,2]

    v = np.einsum("hde,rhe->hdr", Wk, qs)                   # [H,DH,2]
    V = np.zeros((D, 2 * H), np.float32)
    for h in range(H):
        V[64 * h:64 * h + 64, 2 * h] = v[h, :, 0]
        V[64 * h:64 * h + 64, 2 * h + 1] = v[h, :, 1]
    c0 = np.empty(2 * H, np.float32)
    for h in range(H):
        c0[2 * h] = qs[0, h] @ bk[h]
        c0[2 * h + 1] = qs[1, h] @ bk[h]

    Vg = g[:, None] * V
    # augmented score matrix: [Vg | ones | b_map]
    Vaug = np.concatenate(
        [Vg, np.ones((D, 1), np.float32), b_map[:, None]], 1)  # [D, 34]
    consts = dict(
        pg=Vg.sum(0),
        qb=(lb[:, None] * V).sum(0),
        bVg=(b_map[:, None] * Vg).sum(0),
        bmean=b_map.mean(),
        bsq=(b_map ** 2).sum(),
        s_spec=s_spec,
        c0=c0,
    )

    Wd = W.astype(np.float64)
    Wv = (Wd @ Vaug.astype(np.float64)).astype(np.float32)   # [512, 34]
    G = Wd @ Wd.T
    L = np.linalg.cholesky(G).astype(np.float32)             # [512, 512] lower
    return x, Wv, L, consts


def _pack_L(L):
    """[512, 512] lower-tri -> tight rhs pack [128p, 1280] bf16: row-block i
    (L[128i:128i+128, :]) at column offset LOFF[i], width 128*(i+1)."""
    Lb = _bf16(L)
    out = np.zeros((128, LOFF[3] + 512), ml_dtypes.bfloat16)
    for i in range(4):
        n = 128 * (i + 1)
        out[:, LOFF[i]:LOFF[i] + n] = Lb[128 * i:128 * i + 128, 0:n]
    return out


def kernel(**inputs):
    from concourse.bass_utils import run_bass_kernel_spmd

    x, Wv, L, consts = _host_precompute(inputs)

    key = (PRECISION, WARMUP_MMS)
    if key not in _PROG_CACHE:
        _PROG_CACHE[key] = _build_program(PRECISION, WARMUP_MMS)
    nc = _PROG_CACHE[key]

    mixed = PRECISION == "mixed"

    xf = x.reshape(B * N, IN_D)
    xpad = np.zeros((JTOT, IN_D), np.float32)
    xpad[:B * N] = xf

    def pack_x(a, dt):
        # [512, 1024] -> [128p, 2jt, 4i, 512j]
        return np.ascontiguousarray(
            np.asarray(a).astype(dt).reshape(4, 128, 2, 512)
            .transpose(1, 2, 0, 3))

    shared = {
        "lt": _pack_L(L),
        "wv": np.ascontiguousarray(
            _bf16(Wv).reshape(4, 128, NAUG).transpose(1, 0, 2)),
    }
    if mixed:
        # fp8 L row-blocks 2,3 at full 512 width for the DoubleRow matmul
        L8 = _fp8(L)
        shared["lz"] = np.ascontiguousarray(
            np.stack([L8[256:384, :], L8[384:512, :]], 1))

    in_maps = []
    for c in range(NCORES):
        xT = np.ascontiguousarray(xpad[c * JPC:(c + 1) * JPC].T)  # [512, 1024]
        m = dict(shared)
        m["xb"] = pack_x(xT, ml_dtypes.bfloat16)
        if mixed:
            m["xz"] = np.ascontiguousarray(
                pack_x(xT, ml_dtypes.float8_e4m3)[:, :, 2:4, :])
        in_maps.append(m)

    trace = bool(int(os.environ.get("AS_TRACE", "0")))
    res = run_bass_kernel_spmd(nc, in_maps, list(range(NCORES)), trace=trace)
    global LAST_RESULT
    LAST_RESULT = res
    outs = [(np.asarray(r["out"], np.float32), np.asarray(r["outs"], np.float32))
            for r in res.results]

    return _epilogue(outs, consts)


DVE_COLS = (2, 5)


def _sq_from_outs(s):
    # s: [128, 12]: cols 0:8 ACT sum-of-squares; cols 8:12 the DVE jobs'
    # bn_stats (mean, var) pairs -> SQ = 512*(var + mean^2)
    sq = np.array(s[:, 0:8])
    for k, col in enumerate(DVE_COLS):
        mean = s[:, 8 + 2 * k]
        var = s[:, 9 + 2 * k]
        sq[:, col] = 512.0 * (var + mean * mean)
    return sq.T.reshape(JPC)


def _epilogue(outs, consts):
    # outs: per-core ([34, JPC] Y^T, [128, 12] sumsq data: SQ[128g+p])
    yfull = np.concatenate([o[0].T for o in outs], 0)[:B * N]
    SQ = np.concatenate([_sq_from_outs(o[1]) for o in outs], 0)[:B * N]
    Y = yfull[:, 0:32]
    colsum = yfull[:, 32]
    bcross = yfull[:, 33]

    mu = colsum / np.float32(D) + consts["bmean"]
    E2 = (SQ + 2.0 * bcross + consts["bsq"]) / np.float32(D)
    var = E2 - mu ** 2
    rstd = (1.0 / np.sqrt(var + EPS)).astype(np.float32)
    G = Y + consts["bVg"][None]
    sc = (rstd[:, None] * G
          - (rstd * mu)[:, None] * consts["pg"][None]
          + consts["qb"][None] + consts["c0"][None])
    sc = sc.reshape(B, N, H, 2).transpose(0, 2, 3, 1)       # [B,H,2,N]

    scores = np.empty((B, H, 2, S), np.float32)
    scores[:, :, :, 2:] = sc
    scores[:, :, :, 0:2] = consts["s_spec"][None]

    m = scores - scores.max(-1, keepdims=True)
    e = np.exp(m)
    attn = e / e.sum(-1, keepdims=True)
    mm = attn.mean(1)                                       # [B,2,S]
    return (np.ascontiguousarray(mm[:, 0, :]),
            np.ascontiguousarray(mm[:, 1, :]))
